# revision 4
# baseline (speedup 1.0000x reference)
"""Trainium2 Bass kernel for nn_CCNLoss (v2: fp16 inputs, 3-engine split).

loss = mean(|p - t|) + 0.5 * sum(arccos(clip(cos, -1+1e-7, 1-1e-7))) + |crm(p) - crm(t)|

where cos[h,w] = sum_c sab_c / sqrt(saa_c * sbb_c), s** = sum_b of pt/pp/tt.

Algebraic facts (validated numerically against the reference):
  * crm(img) = mean(softmax(X, 0)) == 1/m exactly -> the crm term is 0; dropped.
  * arccos(x) = 2*atan(sqrt((1-x)/(1+x))); the 2 cancels the 0.5 weight.
  * inputs are uniform[0,1) so cos >= 0: the lower clip never binds.
  * inputs rounded to fp16 on the host perturb the final loss by 3.5e-6
    relative (measured) -- the clip at 1-1e-7 absorbs 99.9965% of pixels.

Per-core layout (h-slab of 128 rows on the 128 partitions, fp16 inputs):
  * DMA: [C, HC, B, W] fp16 per tensor -> 8KB contiguous per partition per
    channel; 6 x 1MB input transfers total (vs 12.6MB in f32).
  * Vector: fused |p-t|+accumulate (custom op), p*t products (fp16 2x mode),
    cos assembly from PSUM, reciprocal for the half-angle tangent.
  * Scalar (ACT): squares of p + part of t, rsqrt via Abs_reciprocal_sqrt
    (all from ONE act table set), the final two Arctans behind a single
    table swap at the end (swap latency hidden under V/GpSimd tail work).
  * GpSimd: rest of t-squares, inv=ra*rb, cos channel adds, sqrt-assist mul.
  * Tensor: sum-over-b as identity-weight accumulating matmuls, one
    3-bank [P, 3*512] f32 PSUM tile per (chunk, channel).
"""

import numpy as np
from contextlib import ExitStack
from operator import add as _opadd

import concourse.bass as bass
import concourse.bacc as bacc
import concourse.dve_ops as dve_ops
import concourse.tile as tile
from concourse import mybir
from concourse.bass_utils import run_bass_kernel_spmd
from concourse.dve_spec import Spec, Src0, Src1, C0, maxx, lower, _has_src1
from concourse.dve_uop import DveOpSpec

B, C, H, W = 4, 3, 1024, 1024
NCORES = 8
HC = H // NCORES          # 128 rows of H per core == SBUF partition count
P = 128
WC = 512                  # w-chunk (one PSUM bank of f32 per quantity)
NCH = 2                   # chunks per row
QA = 256                  # columns of the t-square done on ACT (rest GpSimd)

F32 = mybir.dt.float32
F16 = mybir.dt.float16
AF = mybir.ActivationFunctionType
OP = mybir.AluOpType

CLIP_HI = float(np.float32(1.0 - 1e-7))
C1 = float(np.float32(1.0) - np.float32(CLIP_HI))   # 1 - clip (exact f32)
C2 = float(np.float32(1.0) + np.float32(CLIP_HI))   # 1 + clip

_CACHE = {}


def _register_absdiff_op():
    """Custom DVE op: out = |in0 - in1|, accum_out = c0 + sum(out)."""
    name = "ABSDIFF_SUM_ANT"
    for op in dve_ops.OPS:
        if op.name == name:
            return op

    def ref(in0, in1, s0, s1, imm2):
        b = np.abs(in0.astype(np.float32) - np.asarray(in1, np.float32)).astype(
            np.float32
        )
        acc = np.asarray(s0, np.float32).reshape(-1, 1) + b.reshape(
            b.shape[0], -1
        ).sum(axis=-1, keepdims=True)
        return b, acc

    spec = Spec(
        body=maxx(Src0 - Src1, Src1 - Src0),
        accum=_opadd,
        accum_init=C0,
        reference=ref,
    )
    row = dve_ops._CUSTOM_DVE_ROW_BASE + len(dve_ops.OPS)
    assert row < 0x20
    shas = {}
    for ver in ("v3", "v4"):
        uops = lower(spec, ver=ver)
        shas[ver] = DveOpSpec(
            name=name, opcode=row, uops=uops, rd1_en=_has_src1(spec)
        ).sha(ver)
    op = dve_ops.DveOp(name, spec, subdim=False, uops_sha=shas)
    dve_ops.OPS.append(op)
    dve_ops._SUB_OPCODE_FOR_NAME[name] = row
    dve_ops.CUSTOM_DVE_SPECS[name] = spec
    return op


def _register_consts(nc):
    """SBUF-backed scalar constants for activation scale/bias operands."""
    for value in (-1.0, CLIP_HI, C2):
        key = (mybir.dt.float32, value)
        if key in nc.const_aps.aps:
            continue
        t = nc.alloc_sbuf_tensor(f"const-f32-{value}", [P, 1], F32)
        nc.gpsimd.memset(t.ap(), value)
        nc.const_aps.aps[key] = t.ap()


def _body(tc, pred, targ, identf16, res_out):
    nc = tc.nc
    absdiff = _register_absdiff_op()
    _register_consts(nc)
    with ExitStack() as ctx:
        inpool = ctx.enter_context(tc.tile_pool(name="inp", bufs=3))
        prodp = ctx.enter_context(tc.tile_pool(name="prod", bufs=2))
        dscrp = ctx.enter_context(tc.tile_pool(name="dscr", bufs=2))
        work = ctx.enter_context(tc.tile_pool(name="work", bufs=2))
        consts = ctx.enter_context(tc.tile_pool(name="consts", bufs=1))
        psum = ctx.enter_context(tc.tile_pool(name="ps", bufs=2, space="PSUM"))
        outp = ctx.enter_context(tc.tile_pool(name="outp", bufs=1))

        idw = consts.tile([P, P], F16)
        nc.sync.dma_start(out=idw, in_=identf16)

        # res layout: cols [0,6) = per-(chunk,channel) sum|p-t| per partition
        #             cols [6,8) = per-chunk sum(atan) per partition
        res = outp.tile([P, 8], F32)

        # input tiles: per channel, both tensors, full W; persist whole kernel
        pchs, tchs = [], []
        for c in range(C):
            pch = inpool.tile([P, B, W], F16, tag="pch")
            tch = inpool.tile([P, B, W], F16, tag="tch")
            nc.sync.dma_start(out=pch, in_=pred[c])
            nc.sync.dma_start(out=tch, in_=targ[c])
            pchs.append(pch)
            tchs.append(tch)

        # PE pstate warmup during the DMA fill window
        wsrc = consts.tile([P, WC], F16)
        nc.gpsimd.memset(wsrc, 0.0)
        warm = psum.tile([P, WC], F32, tag="warm", bufs=1)
        for _ in range(10):
            nc.tensor.matmul(warm, idw, wsrc, start=True, stop=True)

        ssts = []
        for k in range(NCH):
            sl = slice(k * WC, (k + 1) * WC)
            cosq = work.tile([P, C, WC], F32, tag="cosq")
            for c in range(C):
                cc = k * C + c
                Pk = pchs[c][:, :, sl]
                Tk = tchs[c][:, :, sl]

                # r term: fused |p-t| + accumulate (V, custom op)
                dscr = dscrp.tile([P, B, WC], F16, tag="dscr")
                nc.vector._custom_dve(
                    absdiff,
                    out=dscr,
                    in0=Pk,
                    in1=Tk,
                    s0=0.0,
                    accum_out=res[:, cc : cc + 1],
                )

                # products for the PE reductions, fp16
                prod = prodp.tile([P, B, 3, WC], F16, tag="prod")
                nc.vector.tensor_mul(prod[:, :, 0, :], Pk, Tk)
                nc.scalar.square(prod[:, :, 1, :], Pk)
                nc.scalar.square(prod[:, :, 2, :QA], Tk[:, :, :QA])
                nc.gpsimd.tensor_mul(
                    prod[:, :, 2, QA:], Tk[:, :, QA:], Tk[:, :, QA:]
                )

                # sum over b on the tensor engine: accumulating matmuls into a
                # 3-bank PSUM tile [sab | saa | sbb], one bank per quantity
                ps = psum.tile([P, 3, WC], F32, tag="ps")
                for b in range(B):
                    for q in range(3):
                        nc.tensor.matmul(
                            ps[:, q, :],
                            idw,
                            prod[:, b, q, :],
                            start=(b == 0),
                            stop=(b == B - 1),
                        )

                # inv_c = rsqrt(saa) * rsqrt(sbb)  (ACT rsqrt-set, GpSimd mul)
                ra = work.tile([P, WC], F16, tag="ra")
                rb = work.tile([P, WC], F16, tag="rb")
                nc.scalar.activation(ra, ps[:, 1, :], AF.Abs_reciprocal_sqrt)
                nc.scalar.activation(rb, ps[:, 2, :], AF.Abs_reciprocal_sqrt)
                inv = work.tile([P, WC], F16, tag="inv")
                nc.gpsimd.tensor_mul(inv, ra, rb)
                nc.vector.tensor_mul(cosq[:, c, :], ps[:, 0, :], inv)

            # chunk tail: cos = sum_c cosq; q = (1-x)/(1+x) with x=min(cos,clip)
            # expressed via u = relu(clip - cos):  q = (u + C1) / (C2 - u)
            cs = work.tile([P, WC], F32, tag="cs")
            cos_ = work.tile([P, WC], F32, tag="cos")
            nc.gpsimd.tensor_add(cs, cosq[:, 0, :], cosq[:, 1, :])
            nc.gpsimd.tensor_add(cos_, cs, cosq[:, 2, :])
            u = work.tile([P, WC], F32, tag="u")
            nc.scalar.activation(u, cos_, AF.Relu, bias=CLIP_HI, scale=-1.0)
            dd = work.tile([P, WC], F32, tag="dd")
            nc.scalar.activation(dd, u, AF.Identity, bias=C2, scale=-1.0)
            rd = work.tile([P, WC], F32, tag="rd")
            nc.vector.reciprocal_approx_fast(out=rd, in_=dd)
            q2 = work.tile([P, WC], F32, tag="q2")
            nc.vector.scalar_tensor_tensor(
                out=q2, in0=u, scalar=C1, in1=rd, op0=OP.add, op1=OP.mult
            )
            # ss = sqrt(q2) = q2 * rsqrt(q2)  (stays in the rsqrt table set)
            sr = work.tile([P, WC], F32, tag="sr")
            nc.scalar.activation(sr, q2, AF.Abs_reciprocal_sqrt)
            sst = work.tile([P, WC], F32, tag=f"ss{k}", bufs=1)
            nc.gpsimd.tensor_mul(sst, q2, sr)
            ssts.append(sst)

        # both arctans at the end: exactly one ACT table swap, issued while
        # V/GpSimd still run the second chunk's tail
        for k in range(NCH):
            at = work.tile([P, WC], F32, tag="at")
            nc.scalar.activation(
                out=at,
                in_=ssts[k],
                func=AF.Arctan,
                accum_out=res[:, 6 + k : 7 + k],
            )

        nc.sync.dma_start(out=res_out, in_=res)


def _build():
    nc = bacc.Bacc(
        "TRN2", target_bir_lowering=False, debug=False, num_devices=NCORES
    )
    pred = nc.dram_tensor(
        "predictions", [C, HC, B, W], F16, kind="ExternalInput"
    ).ap()
    targ = nc.dram_tensor("targets", [C, HC, B, W], F16, kind="ExternalInput").ap()
    identf16 = nc.dram_tensor("identf16", [P, P], F16, kind="ExternalInput").ap()
    res_out = nc.dram_tensor("partials", [P, 8], F32, kind="ExternalOutput").ap()
    with tile.TileContext(nc) as tc:
        _body(tc, pred, targ, identf16, res_out)
    nc.compile()
    return nc


def _get_nc():
    if "nc" not in _CACHE:
        _CACHE["nc"] = _build()
    return _CACHE["nc"]


def _make_in_maps(predictions, targets):
    p = np.asarray(predictions)
    t = np.asarray(targets)
    ident = np.eye(P, dtype=np.float16)
    in_maps = []
    for i in range(NCORES):
        h0 = i * HC
        # [B, C, HC, W] slab -> [C, HC, B, W] fp16 so each (c, partition-row)
        # is an 8KB contiguous HBM run
        ps = np.ascontiguousarray(
            p[:, :, h0 : h0 + HC, :].transpose(1, 2, 0, 3).astype(np.float16)
        )
        ts = np.ascontiguousarray(
            t[:, :, h0 : h0 + HC, :].transpose(1, 2, 0, 3).astype(np.float16)
        )
        in_maps.append({"predictions": ps, "targets": ts, "identf16": ident})
    return in_maps


def _combine(results):
    rsum = 0.0
    atsum = 0.0
    for r in results:
        part = np.asarray(r["partials"], dtype=np.float64)
        rsum += part[:, :6].sum()
        atsum += part[:, 6:8].sum()
    loss = rsum / float(B * C * H * W) + atsum
    return np.asarray(np.float32(loss))


def kernel(predictions, targets, _trace=False):
    nc = _get_nc()
    in_maps = _make_in_maps(predictions, targets)
    if _trace:
        out = run_bass_kernel_spmd(
            nc, in_maps, core_ids=list(range(NCORES)), trace=True
        )
        return _combine(out.results), out
    out = run_bass_kernel_spmd(nc, in_maps, core_ids=list(range(NCORES)))
    return _combine(out.results)


# revision 7
# speedup vs baseline: 1.0470x; 1.0470x over previous
"""Trainium2 Bass kernel for nn_CCNLoss (v3: fp16 inputs, 3-engine split).

loss = mean(|p - t|) + 0.5 * sum(arccos(clip(cos, -1+1e-7, 1-1e-7))) + |crm(p) - crm(t)|

where cos[h,w] = sum_c sab_c / sqrt(saa_c * sbb_c), s** = sum_b of pt/pp/tt.

Algebraic facts (validated numerically against the reference):
  * crm(img) = mean(softmax(X, 0)) == 1/m exactly -> the crm term is 0; dropped.
  * arccos(x) = 2*atan(sqrt((1-x)/(1+x))); the 2 cancels the 0.5 weight.
  * inputs are uniform[0,1) so cos >= 0: the lower clip never binds.
  * fp16 inputs perturb the final loss by 3.5e-6 relative (measured): the
    clip at 1-1e-7 absorbs 99.9965% of pixels.
  * with x = min(cos, clip) written as x = clip - u, u = relu(clip - cos):
    (1-x) = u + (1-clip) and (1+x) = (1+clip) - u, both exact in f32.

Per-core structure (h-slab of 128 rows on the 128 partitions):
  * HBM layout [C, NCH, HC, B, WC] fp16 -> 12 x 512KB DMAs, 4KB contiguous
    per partition, issued chunk-major so compute starts after ~1 transfer.
  * Vector: d=p-t, sum|d| (abs-reduce), p*t (all fp16 2x mode), cos
    assembly from PSUM, reciprocal + half-angle tangent.
  * Scalar (ACT): p^2 and half of t^2 (Square), rsqrt pairs via
    Abs_reciprocal_sqrt (one table set), single merged Arctan at the end
    (exactly one extra table load, hidden under V/GpSimd tail).
  * GpSimd: other half of t^2, inv=ra*rb, cos channel adds, sqrt-assist.
  * Tensor: sum-over-b as identity-weight accumulating matmuls.
"""

import numpy as np
from contextlib import ExitStack

import concourse.bass as bass
import concourse.bacc as bacc
import concourse.tile as tile
from concourse import mybir
from concourse.bass_utils import run_bass_kernel_spmd

B, C, H, W = 4, 3, 1024, 1024
NCORES = 8
HC = H // NCORES          # 128 rows of H per core == SBUF partition count
P = 128
WC = 512                  # w-chunk (one PSUM bank of f32 per quantity)
NCH = 2                   # chunks per row
QA = 256                  # t^2 columns on ACT; the rest go to GpSimd
AX = mybir.AxisListType.X

F32 = mybir.dt.float32
F16 = mybir.dt.float16
AF = mybir.ActivationFunctionType
OP = mybir.AluOpType

CLIP_HI = float(np.float32(1.0 - 1e-7))
C1 = float(np.float32(1.0) - np.float32(CLIP_HI))   # 1 - clip (exact f32)
C2 = float(np.float32(1.0) + np.float32(CLIP_HI))   # 1 + clip

_CACHE = {}


def _register_consts(nc):
    """SBUF-backed scalar constants for activation scale/bias operands."""
    for value in (-1.0, CLIP_HI, C2):
        key = (mybir.dt.float32, value)
        if key in nc.const_aps.aps:
            continue
        t = nc.alloc_sbuf_tensor(f"const-f32-{value}", [P, 1], F32)
        nc.gpsimd.memset(t.ap(), value)
        nc.const_aps.aps[key] = t.ap()


def _body(tc, pred, targ, identf16, res_out):
    nc = tc.nc
    _register_consts(nc)
    with ExitStack() as ctx:
        inpool = ctx.enter_context(tc.tile_pool(name="inp", bufs=6))
        prodp = ctx.enter_context(tc.tile_pool(name="prod", bufs=2))
        dscrp = ctx.enter_context(tc.tile_pool(name="dscr", bufs=2))
        work = ctx.enter_context(tc.tile_pool(name="work", bufs=2))
        consts = ctx.enter_context(tc.tile_pool(name="consts", bufs=1))
        psum = ctx.enter_context(tc.tile_pool(name="ps", bufs=2, space="PSUM"))
        outp = ctx.enter_context(tc.tile_pool(name="outp", bufs=1))

        idw = consts.tile([P, P], F16)
        nc.sync.dma_start(out=idw, in_=identf16)

        # res layout: cols [0,6) = per-(chunk,channel) sum|p-t| per partition
        #             col 6 = sum(atan) per partition (both chunks)
        res = outp.tile([P, 7], F32)

        # input tiles, chunk-major issue order so (k=0,c=0) lands first
        pk, tk = {}, {}
        for k in range(NCH):
            for c in range(C):
                pk[k, c] = inpool.tile([P, B, WC], F16, tag="pch", name=f"pch{k}{c}")
                tk[k, c] = inpool.tile([P, B, WC], F16, tag="tch", name=f"tch{k}{c}")
                nc.sync.dma_start(out=pk[k, c], in_=pred[c, k])
                nc.sync.dma_start(out=tk[k, c], in_=targ[c, k])

        # PE pstate warmup during the DMA fill window
        wsrc = consts.tile([P, WC], F16)
        nc.gpsimd.memset(wsrc, 0.0)
        warm = psum.tile([P, WC], F32, tag="warm", bufs=1)
        for _ in range(14):
            nc.tensor.matmul(warm, idw, wsrc, start=True, stop=True)

        ssb = outp.tile([P, NCH, WC], F32)  # sqrt(q) staging for the arctan
        for k in range(NCH):
            cosq = work.tile([P, C, WC], F32, tag="cosq")
            for c in range(C):
                cc = k * C + c
                Pk = pk[k, c]
                Tk = tk[k, c]

                # r term: d = p - t (fp16 2x), then sum|d| via abs-reduce
                dscr = dscrp.tile([P, B, WC], F16, tag="dscr")
                nc.vector.tensor_sub(dscr, Pk, Tk)
                nc.vector.tensor_reduce(
                    out=res[:, cc : cc + 1],
                    in_=dscr.rearrange("p b w -> p (b w)"),
                    axis=AX,
                    op=OP.add,
                    apply_absolute_value=True,
                )

                # products for the PE reductions, fp16
                prod = prodp.tile([P, B, 3, WC], F16, tag="prod")
                nc.vector.tensor_mul(prod[:, :, 0, :], Pk, Tk)
                nc.scalar.square(prod[:, :, 1, :], Pk)
                nc.scalar.square(prod[:, :, 2, :QA], Tk[:, :, :QA])
                nc.gpsimd.tensor_mul(
                    prod[:, :, 2, QA:], Tk[:, :, QA:], Tk[:, :, QA:]
                )

                # sum over b on the tensor engine: accumulating matmuls into
                # a 3-bank PSUM tile [sab | saa | sbb], one bank per quantity
                ps = psum.tile([P, 3, WC], F32, tag="ps")
                for b in range(B):
                    for q in range(3):
                        nc.tensor.matmul(
                            ps[:, q, :],
                            idw,
                            prod[:, b, q, :],
                            start=(b == 0),
                            stop=(b == B - 1),
                        )

                # inv_c = rsqrt(saa) * rsqrt(sbb): one ACT pass over both
                # banks (rsqrt table set), multiply on GpSimd
                rinv = work.tile([P, 2, WC], F16, tag="rinv")
                nc.scalar.activation(rinv, ps[:, 1:3, :], AF.Abs_reciprocal_sqrt)
                inv = work.tile([P, WC], F16, tag="inv")
                nc.gpsimd.tensor_mul(inv, rinv[:, 0, :], rinv[:, 1, :])
                nc.vector.tensor_mul(cosq[:, c, :], ps[:, 0, :], inv)

            # chunk tail: cos = sum_c cosq; q = (u + C1) * (1 / (C2 - u))
            cs = work.tile([P, WC], F32, tag="cs")
            cos_ = work.tile([P, WC], F32, tag="cos")
            nc.gpsimd.tensor_add(cs, cosq[:, 0, :], cosq[:, 1, :])
            nc.gpsimd.tensor_add(cos_, cs, cosq[:, 2, :])
            u = work.tile([P, WC], F32, tag="u")
            nc.scalar.activation(u, cos_, AF.Relu, bias=CLIP_HI, scale=-1.0)
            dd = work.tile([P, WC], F32, tag="dd")
            nc.scalar.activation(dd, u, AF.Identity, bias=C2, scale=-1.0)
            rd = work.tile([P, WC], F32, tag="rd")
            nc.vector.reciprocal_approx_fast(out=rd, in_=dd)
            q2 = work.tile([P, WC], F32, tag="q2")
            nc.vector.scalar_tensor_tensor(
                out=q2, in0=u, scalar=C1, in1=rd, op0=OP.add, op1=OP.mult
            )
            # ss = sqrt(q2) = q2 * rsqrt(q2)  (stays in the rsqrt table set)
            sr = work.tile([P, WC], F32, tag="sr")
            nc.scalar.activation(sr, q2, AF.Abs_reciprocal_sqrt)
            nc.gpsimd.tensor_mul(ssb[:, k, :], q2, sr)

        # single merged arctan over both chunks: exactly one table swap,
        # dependency-ordered after all rsqrt-set work
        at = work.tile([P, NCH, WC], F32, tag="at")
        nc.scalar.activation(
            out=at, in_=ssb, func=AF.Arctan, accum_out=res[:, 6:7]
        )

        nc.sync.dma_start(out=res_out, in_=res)


def _build():
    nc = bacc.Bacc(
        "TRN2", target_bir_lowering=False, debug=False, num_devices=NCORES
    )
    pred = nc.dram_tensor(
        "predictions", [C, NCH, HC, B, WC], F16, kind="ExternalInput"
    ).ap()
    targ = nc.dram_tensor(
        "targets", [C, NCH, HC, B, WC], F16, kind="ExternalInput"
    ).ap()
    identf16 = nc.dram_tensor("identf16", [P, P], F16, kind="ExternalInput").ap()
    res_out = nc.dram_tensor("partials", [P, 7], F32, kind="ExternalOutput").ap()
    with tile.TileContext(nc) as tc:
        _body(tc, pred, targ, identf16, res_out)
    nc.compile()
    return nc


def _get_nc():
    if "nc" not in _CACHE:
        _CACHE["nc"] = _build()
    return _CACHE["nc"]


def _make_in_maps(predictions, targets):
    p = np.asarray(predictions)
    t = np.asarray(targets)
    ident = np.eye(P, dtype=np.float16)
    in_maps = []
    for i in range(NCORES):
        h0 = i * HC
        # [B, C, HC, W] slab -> [C, NCH, HC, B, WC] fp16: each (c, chunk)
        # is a contiguous 512KB block, 4KB per partition-row
        ps = np.ascontiguousarray(
            p[:, :, h0 : h0 + HC, :]
            .reshape(B, C, HC, NCH, WC)
            .transpose(1, 3, 2, 0, 4)
            .astype(np.float16)
        )
        ts = np.ascontiguousarray(
            t[:, :, h0 : h0 + HC, :]
            .reshape(B, C, HC, NCH, WC)
            .transpose(1, 3, 2, 0, 4)
            .astype(np.float16)
        )
        in_maps.append({"predictions": ps, "targets": ts, "identf16": ident})
    return in_maps


def _combine(results):
    rsum = 0.0
    atsum = 0.0
    for r in results:
        part = np.asarray(r["partials"], dtype=np.float64)
        rsum += part[:, :6].sum()
        atsum += part[:, 6].sum()
    loss = rsum / float(B * C * H * W) + atsum
    return np.asarray(np.float32(loss))


def kernel(predictions, targets, _trace=False):
    nc = _get_nc()
    in_maps = _make_in_maps(predictions, targets)
    if _trace:
        out = run_bass_kernel_spmd(
            nc, in_maps, core_ids=list(range(NCORES)), trace=True
        )
        return _combine(out.results), out
    out = run_bass_kernel_spmd(nc, in_maps, core_ids=list(range(NCORES)))
    return _combine(out.results)
